# revision 1
# baseline (speedup 1.0000x reference)
"""CircleLoss kernel for Trainium2 (Bass/Tile), SPMD across 8 NeuronCores.

Math (s=32, m=0.25, B=8192, D=128):
    u = cos-sim matrix (row-normalized feats @ feats.T)
    p-side: since u <= 1 < 1+m, relu(1+m-u) = 1+m-u always, and
        expo_p = -s*(1+m-u)*(u-(1-m)) = s*(1-u)^2 - s*m^2 = s*(1-u)^2 - 2
    n-side: r = relu(u+m), expo_n = s*r*(u-m) = s*(r-m)^2 - s*m^2  (exact
        also at r=0: exp(0)=1 matches reference's exp(0)=1? no - reference
        gives exp(0)=1 only for masked-in entries; masking handled below)
    Masking: one bf16 mask mn = 4*(label_i != label_j), built by a DVE
    is_equal against partition-replicated labels, is folded additively into
    both exp arguments (p: t = w - mn with bias -2; n: t2 = w2 + mn with
    bias -130 = -(4*s + 2)) so masked-out entries underflow to 0.
    Diagonal: self-sim u_ii ~= 1 and same_ii=1, so the unmasked p-sum gains
    exactly exp(-2) per row; subtracted as a constant on the host.

Implementation: per core, 8192x128 feats are normalized + PE-transposed into
fT [128, 8192] f32r once; the [1024, 8192] slab is processed in 32 chunks of
[128 rows x 2048 cols]: 4 f32r matmuls fill a 4-bank PSUM tile, a single
PSUM read converts u to bf16 (alternating ACT/DVE), and the rest of the
chain runs as cheap bf16 DVE 2x-mode ops (p-square alternates ACT Square /
DVE+Pool multiply), with ACT Exp+row-accumulate producing per-row partial
sums. Emission is software-pipelined in 3 skewed stages so no engine
head-of-line blocks. Measured ~220-275us/core on HW (~55us prep + ~165-220us
main), rel err vs reference ~5e-4 (bf16 intermediates).

Sharding: core c owns rows [c*1024, (c+1)*1024). Each core loads the full
feats, normalizes+transposes locally (cheaper than any collective), computes
its [1024, 8192] slab, and returns per-row partial sums P,N packed [128,16].
Host finishes: P -= e^-2, loss = mean(log1p(P*N)).
"""

import os
import numpy as np
from contextlib import ExitStack

import concourse.bass as bass
import concourse.bacc as bacc
import concourse.tile as tile
import concourse.mybir as mybir
from concourse.bass_utils import run_bass_kernel_spmd

B, D, NCORES = 8192, 128, 8
BL = B // NCORES          # 1024 rows per core
S, M = 32.0, 0.25
MASKV = 4.0               # mask magnitude: exp arg shifts by -S*MASKV = -128
NCH = 512                 # similarity n-chunk width (1 PSUM bank of f32)
N_NCK = B // NCH          # 16
N_MC = BL // 128          # 8 row-chunks of 128 per core
F32 = mybir.dt.float32
F32R = mybir.dt.float32r
BF16 = mybir.dt.bfloat16
AF = mybir.ActivationFunctionType
ALU = mybir.AluOpType

_NC_CACHE = None
LAST_RESULTS = None       # BassKernelResults of the most recent run


def _register_consts(nc, values):
    # Arbitrary activation biases need a const AP; mimic Bass.__init__'s
    # register_const_ap (memset before Tile's instruction stream + barrier).
    for v in values:
        key = (F32, float(v))
        if key in nc.const_aps.aps:
            continue
        t = nc.alloc_sbuf_tensor(f"const-f32-{v}", [128, 1], F32)
        nc.gpsimd.memset(t.ap(), float(v))
        nc.const_aps.aps[key] = t.ap()
    nc.all_engine_barrier()


def _build_nc(loops=1, prep=1):
    nc = bacc.Bacc(
        "TRN2", target_bir_lowering=False, debug=False, num_devices=NCORES
    )
    _register_consts(nc, [-2.0, -130.0])
    feats = nc.dram_tensor("feats", [B, D], F32, kind="ExternalInput").ap()
    floc = nc.dram_tensor("floc", [BL, D], F32, kind="ExternalInput").ap()
    lab_all = nc.dram_tensor("lab_all", [1, B], BF16, kind="ExternalInput").ap()
    lab_loc = nc.dram_tensor("lab_loc", [128, N_MC], F32, kind="ExternalInput").ap()
    ident = nc.dram_tensor("ident", [128, 128], F32, kind="ExternalInput").ap()
    out = nc.dram_tensor("out", [128, 2 * N_MC], F32, kind="ExternalOutput").ap()

    with tile.TileContext(nc) as tc, ExitStack() as ctx:
        persist = ctx.enter_context(tc.tile_pool(name="persist", bufs=1))
        ft_pool = ctx.enter_context(tc.tile_pool(name="ft", bufs=3))
        sm_pool = ctx.enter_context(tc.tile_pool(name="sm", bufs=4))

        fT = persist.tile([128, B], F32R, name="fT")
        fTloc = persist.tile([128, BL], F32R, name="fTloc")
        labR = persist.tile([128, B], BF16, name="labR")
        labL = persist.tile([128, N_MC], F32, name="labL")
        idn = persist.tile([128, 128], F32, name="idn")
        stats = persist.tile([128, 2 * N_MC], F32, name="stats")

        nc.sync.dma_start(out=idn[:], in_=ident)
        nc.sync.dma_start(out=labR[:], in_=lab_all.to_broadcast((128, B)))
        nc.sync.dma_start(out=labL[:], in_=lab_loc)

        def norm_transpose(tp_pool, src_dram, n_rows, dst):
            """dst[:, i] = src[i, :] / ||src[i, :]|| for i in range(n_rows)."""
            for t in range(n_rows // 128):
                ftile = ft_pool.tile([128, D], F32, tag="ftile")
                nc.sync.dma_start(
                    out=ftile[:], in_=src_dram[t * 128 : (t + 1) * 128, :]
                )
                sq = ft_pool.tile([128, D], F32, tag="sq")
                nc.gpsimd.tensor_mul(sq[:], ftile[:], ftile[:])
                ssq = sm_pool.tile([128, 1], F32, tag="ssq")
                nc.vector.tensor_reduce(
                    ssq[:], sq[:], axis=mybir.AxisListType.X, op=ALU.add
                )
                nrm = sm_pool.tile([128, 1], F32, tag="nrm")
                nc.scalar.activation(nrm[:], ssq[:], AF.Sqrt)
                inv = sm_pool.tile([128, 1], F32, tag="inv")
                nc.vector.reciprocal(inv[:], nrm[:])
                fnorm = ft_pool.tile([128, D], F32, tag="fnorm")
                nc.vector.tensor_scalar_mul(fnorm[:], ftile[:], inv[:])
                pt = tp_pool.tile([128, 128], F32, tag="pt")
                nc.tensor.transpose(pt[:], fnorm[:], idn[:])
                if t % 2 == 0:
                    nc.vector.tensor_copy(dst[:, t * 128 : (t + 1) * 128], pt[:])
                else:
                    nc.scalar.copy(dst[:, t * 128 : (t + 1) * 128], pt[:])

        if prep:
            with tc.tile_pool(name="tp", bufs=2, space="PSUM") as tp_pool:
                for _prep_rep in range(prep):
                    norm_transpose(tp_pool, floc, BL, fTloc)
                    norm_transpose(tp_pool, feats, B, fT)

        # Main loop v3. EW=2048-wide elementwise (4 matmul quarters per
        # 4-bank PSUM tile). One bf16 mask mn = 4*(label_i != label_j) kills
        # both sides: p-arg = S*(w - mn) - 2 (w=(1-u)^2), n-arg =
        # S*(w2 + mn) - 130 (w2 = max(u,-m)^2; keep needs mn=4: -130+128=-2).
        # bf16 intermediates give DVE its 2x mode on all SBUF tensor-tensor
        # ops; the p-square alternates ACT Square / DVE STT (u^2-2u, bias
        # +30) 3:1 to balance engine load. Pool builds the mask.
        EW = 4 * NCH              # 2048
        N_EW = B // EW            # 4
        ps_pool = ctx.enter_context(tc.tile_pool(name="ps", bufs=2, space="PSUM"))
        el_pool = ctx.enter_context(tc.tile_pool(name="el", bufs=int(os.environ.get("ELBUFS", "4"))))
        ex_pool = ctx.enter_context(tc.tile_pool(name="ex", bufs=int(os.environ.get("EXBUFS", "2"))))
        st_pool = ctx.enter_context(tc.tile_pool(name="st", bufs=2))

        chunks = [(mc, ew) for mc in range(N_MC) for ew in range(N_EW)]
        T = len(chunks)
        live = {}
        pstats, nstats = {}, {}
        pairt = {}

        def s0(c):
            mc, ew = chunks[c]
            if ew == 0:
                pstats[mc] = st_pool.tile([128, N_EW // 2], F32, tag="pstat", name="pstat")
                nstats[mc] = st_pool.tile([128, N_EW // 2], F32, tag="nstat", name="nstat")
            lhs_f = fTloc[:, mc * 128 : (mc + 1) * 128]
            ps = ps_pool.tile([128, EW], F32, tag="ps")
            for h in range(4):
                nsl = slice(ew * EW + h * NCH, ew * EW + (h + 1) * NCH)
                hsl = slice(h * NCH, (h + 1) * NCH)
                nc.tensor.matmul(
                    ps[:, hsl], lhs_f, fT[:, nsl], start=True, stop=True
                )
            esl = slice(ew * EW, (ew + 1) * EW)
            mn = el_pool.tile([128, EW], BF16, tag="mn")
            nc.vector.tensor_scalar(
                mn[:], labR[:, esl], labL[:, mc : mc + 1], MASKV,
                op0=ALU.not_equal, op1=ALU.mult,
            )
            # single PSUM read: u16 = bf16(u); everything downstream runs
            # in DVE 2x mode. Copy engine alternates ACT/DVE to balance.
            u16 = el_pool.tile([128, EW], BF16, tag="u16")
            _CM = int(os.environ.get("COPYMOD", "4"))
            if c % _CM != 0:
                nc.scalar.copy(u16[:], ps[:])
            else:
                nc.vector.tensor_copy(u16[:], ps[:])
            live[c] = [mn, u16]

        def s1(c):
            mn, u16 = live[c]
            mc, ew = chunks[c]
            if ew % 2 == 0:
                pairt[mc, ew // 2] = (
                    el_pool.tile([128, 2 * EW], BF16, tag="t", name="t", bufs=2),
                    el_pool.tile([128, 2 * EW], BF16, tag="t2", name="t2", bufs=2),
                )
            tp, tp2 = pairt[mc, ew // 2]
            hsl = slice((ew % 2) * EW, (ew % 2 + 1) * EW)
            # p: w = (1-u)^2;  t = w - mn   (exp bias -2)
            w = el_pool.tile([128, EW], BF16, tag="w", bufs=2)
            if c % 2 == 0:
                nc.scalar.activation(w[:], u16[:], AF.Square, bias=1.0, scale=-1.0)
            else:
                a = el_pool.tile([128, EW], BF16, tag="a", bufs=2)
                nc.vector.tensor_scalar(
                    a[:], u16[:], -1.0, 1.0, op0=ALU.mult, op1=ALU.add
                )
                nc.gpsimd.tensor_mul(w[:], a[:], a[:])
            nc.vector.tensor_sub(tp[:, hsl], w[:], mn[:])
            # n: r = max(u,-m); w2 = r^2; t2 = w2 + mn  (exp bias -130)
            r = el_pool.tile([128, EW], BF16, tag="r", bufs=2)
            nc.vector.tensor_scalar(r[:], u16[:], -M, None, op0=ALU.max)
            w2 = el_pool.tile([128, EW], BF16, tag="w2", bufs=2)
            nc.vector.tensor_mul(w2[:], r[:], r[:])
            nc.vector.tensor_add(tp2[:, hsl], w2[:], mn[:])
            live[c] = None

        def s2(c):
            mc, ew = chunks[c]
            live.pop(c)
            if ew % 2 == 0:
                return
            t, t2 = pairt.pop((mc, ew // 2))
            pexp = ex_pool.tile([128, 2 * EW], F32, tag="escr")
            nc.scalar.activation(
                pexp[:], t[:], AF.Exp, bias=-2.0, scale=S,
                accum_out=pstats[mc][:, ew // 2 : ew // 2 + 1],
            )
            nexp = ex_pool.tile([128, 2 * EW], F32, tag="escr")
            nc.scalar.activation(
                nexp[:], t2[:], AF.Exp, bias=-130.0, scale=S,
                accum_out=nstats[mc][:, ew // 2 : ew // 2 + 1],
            )
            if ew == N_EW - 1:
                nc.vector.tensor_reduce(
                    stats[:, mc : mc + 1], pstats[mc][:],
                    axis=mybir.AxisListType.X, op=ALU.add,
                )
                nc.vector.tensor_reduce(
                    stats[:, N_MC + mc : N_MC + mc + 1], nstats[mc][:],
                    axis=mybir.AxisListType.X, op=ALU.add,
                )

        if loops == 0:
            nc.gpsimd.memset(stats[:], 0.0)
        import os as _os
        _SK = int(_os.environ.get("SKEW", "3"))
        for rep in range(loops):
            live.clear(); pstats.clear(); nstats.clear(); pairt.clear()
            for c in range(T + _SK):
                if c < T:
                    s0(c)
                if 1 <= c and c - 1 < T:
                    s1(c - 1)
                if _SK <= c and c - _SK < T:
                    s2(c - _SK)
        nc.sync.dma_start(out=out, in_=stats[:])
    nc.compile()
    return nc


def kernel(feats, labels):
    global _NC_CACHE, LAST_RESULTS
    feats = np.ascontiguousarray(np.asarray(feats), dtype=np.float32)
    labels = np.asarray(labels).reshape(-1)
    import ml_dtypes
    lab_bf = labels.astype(ml_dtypes.bfloat16).reshape(1, -1)
    ident = np.eye(128, dtype=np.float32)

    if _NC_CACHE is None:
        _NC_CACHE = _build_nc()
    nc = _NC_CACHE

    in_maps = []
    for c in range(NCORES):
        sl = slice(c * BL, (c + 1) * BL)
        in_maps.append({
            "feats": feats,
            "floc": np.ascontiguousarray(feats[sl]),
            "lab_all": lab_bf,
            "lab_loc": np.ascontiguousarray(
                labels[sl].reshape(N_MC, 128).T.astype(np.float32)
            ),
            "ident": ident,
        })
    res = run_bass_kernel_spmd(
        nc, in_maps, list(range(NCORES)),
        trace=bool(os.environ.get("KERNEL_TRACE")),
    )
    LAST_RESULTS = res

    P_parts, N_parts = [], []
    for c in range(NCORES):
        st = res.results[c]["out"]            # [128, 16]
        P_parts.append(st[:, :N_MC].T.reshape(-1))    # row g=mc*128+p
        N_parts.append(st[:, N_MC:].T.reshape(-1))
    P = np.concatenate(P_parts) - np.float32(np.exp(-2.0))
    N = np.concatenate(N_parts)
    loss_rows = np.log1p((P.astype(np.float32) * N.astype(np.float32)))
    return np.float32(np.mean(loss_rows))

